# revision 4
# baseline (speedup 1.0000x reference)
"""Sparse attention (template/search) Trainium2 Bass kernel.

Reference computation (B=64, N=320, C=768, H=12, D=64, num_t=64, num_s=256):
    qkv = x @ w_qkv.T + b_qkv           -> split to q, k, v per head
    template tokens 0:64   attend to tokens 0:64
    search   tokens 64:320 attend to all 320 tokens
    out = attn_out @ w_proj.T + b_proj

Strategy: data-parallel over batch across 8 NeuronCores (8 batches each).
All layout transposes happen on the host (numpy):
  - x is fed transposed (xT [768, 320] per batch) so the contraction dim is on
    SBUF partitions for the qkv projection.
  - w_qkv/w_proj are fed transposed; the v-weights are interleaved per head
    with an extra "ones" column (stride 65) so the PV matmul produces the
    softmax denominators in the same PSUM tile as the attention output.
On-device dataflow per (batch, head):
  STk   = kT[d, kchunk].T @ qT[d, :]        (scores transposed, k on partitions)
  PT    = exp(STk * 0.125)                  (ScalarE, PSUM -> SBUF, fp32r)
  PV    = vaug[k, 65].T @ PT[k, q]          -> [65, 320]: rows 0:64 = attn outT,
                                               row 64 = colsums (ones column)
  recip = 1/colsums ; bcast = ones[1,64].T @ recip (PE broadcast along partitions)
  aT    = PV[0:64] * bcast                  (VectorE, writes fp32r aT tile)
Projection: out[t, co] = aT[c, t].T @ w_projT[c, co] (+bias via rank-1 matmul).
All matmuls use float32r (full PE rate at N>=256, ~1e-4 rel err).
"""

import sys

sys.path.insert(0, "/opt/trn_rl_repo")

import numpy as np

B, N, C = 64, 320, 768
H, D = 12, 64
NT, NS = 64, 256
NCORES = 8
BC = B // NCORES  # batches per core
CCH = C // 128  # 6 contraction chunks
QK_TILES = (2 * C) // 128  # 12 co-tiles covering q and k sections
TCH = [(0, 128), (128, 128), (256, 64)]  # token chunks (t or k)
VW = H * 65  # 780: v width incl. ones columns
NPH = VW // 2  # 390: vnat free-dim half
PH = C // 2  # 384: proj free-dim half

_CACHE = {}


def _build():
    import concourse.bacc as bacc
    import concourse.mybir as mybir
    import concourse.tile as tile

    F32 = mybir.dt.float32
    F32R = mybir.dt.float32r
    EXP = mybir.ActivationFunctionType.Exp

    nc = bacc.Bacc("TRN2")

    d_xt = nc.dram_tensor("xt", [BC, C, N], F32R, kind="ExternalInput")
    d_wqk = nc.dram_tensor("wqk", [C, 2 * C], F32R, kind="ExternalInput")
    d_wv = nc.dram_tensor("wv", [C, VW], F32R, kind="ExternalInput")
    d_wp = nc.dram_tensor("wp", [C, C], F32R, kind="ExternalInput")
    d_bqk = nc.dram_tensor("bqk", [128, QK_TILES], F32, kind="ExternalInput")
    d_bv = nc.dram_tensor("bv", [1, VW], F32R, kind="ExternalInput")
    d_bp = nc.dram_tensor("bp", [1, C], F32R, kind="ExternalInput")
    d_ones = nc.dram_tensor("ones", [1, 128], F32R, kind="ExternalInput")
    d_out = nc.dram_tensor("out", [BC, N, C], F32, kind="ExternalOutput")

    with tile.TileContext(nc) as tc:
        with (
            tc.tile_pool(name="const", bufs=1) as cp,
            tc.tile_pool(name="work", bufs=2) as wp,
            tc.tile_pool(name="psum", bufs=2, space="PSUM") as pp,
        ):
            # ---- resident weights ----
            wqk_sb = []
            wv_sb = []
            wp_sb = []
            for c in range(CCH):
                t_wqk = cp.tile([128, 2 * C], F32R, name=f"wqk{c}", tag=f"wqk{c}")
                nc.sync.dma_start(t_wqk[:], d_wqk[c * 128 : (c + 1) * 128, :])
                wqk_sb.append(t_wqk)
                t_wv = cp.tile([128, VW], F32R, name=f"wv{c}", tag=f"wv{c}")
                nc.sync.dma_start(t_wv[:], d_wv[c * 128 : (c + 1) * 128, :])
                wv_sb.append(t_wv)
                t_wp = cp.tile([128, C], F32R, name=f"wp{c}", tag=f"wp{c}")
                nc.sync.dma_start(t_wp[:], d_wp[c * 128 : (c + 1) * 128, :])
                wp_sb.append(t_wp)
            bqk_sb = cp.tile([128, QK_TILES], F32, name="bqk", tag="bqk")
            nc.sync.dma_start(bqk_sb[:], d_bqk[:])
            bv_sb = cp.tile([1, VW], F32R, name="bv", tag="bv")
            nc.sync.dma_start(bv_sb[:], d_bv[:])
            bp_sb = cp.tile([1, C], F32R, name="bp", tag="bp")
            nc.sync.dma_start(bp_sb[:], d_bp[:])
            ones_sb = cp.tile([1, 128], F32R, name="ones", tag="ones")
            nc.sync.dma_start(ones_sb[:], d_ones[:])

            for b in range(BC):
                # ---- load xT for this batch ----
                xt_sb = []
                for c in range(CCH):
                    t_xt = wp.tile([128, N], F32R, name=f"xt{b}_{c}", tag="xt", bufs=12)
                    nc.sync.dma_start(t_xt[:], d_xt[b, c * 128 : (c + 1) * 128, :])
                    xt_sb.append(t_xt)

                # ---- qkT projection: [co_tile, 128, N] for q,k sections ----
                qk_sb = []
                for j in range(QK_TILES):
                    ps = pp.tile([128, N], F32, name=f"psqk{b}_{j}", tag="pmm", bufs=2)
                    for c in range(CCH):
                        nc.tensor.matmul(
                            ps[:],
                            wqk_sb[c][:, j * 128 : (j + 1) * 128],
                            xt_sb[c][:],
                            start=(c == 0),
                            stop=(c == CCH - 1),
                        )
                    t_qk = wp.tile([128, N], F32R, name=f"qk{b}_{j}", tag="qkt", bufs=24)
                    nc.any.tensor_scalar_add(t_qk[:], ps[:], bqk_sb[:, j : j + 1])
                    qk_sb.append(t_qk)

                # ---- v natural (+ones cols): vaug [t, 780] per t-chunk ----
                vaug_sb = []
                for ti, (t0, tl) in enumerate(TCH):
                    t_v = wp.tile([tl, VW], F32R, name=f"vaug{b}_{ti}", tag="vaug", bufs=6)
                    for nh in range(2):
                        ps = pp.tile([tl, NPH], F32, name=f"psv{b}_{ti}_{nh}", tag="pmm", bufs=2)
                        for c in range(CCH):
                            nc.tensor.matmul(
                                ps[:],
                                xt_sb[c][:, t0 : t0 + tl],
                                wv_sb[c][:, nh * NPH : (nh + 1) * NPH],
                                start=(c == 0),
                                stop=False,
                            )
                        nc.tensor.matmul(
                            ps[:],
                            ones_sb[:, 0:tl],
                            bv_sb[:, nh * NPH : (nh + 1) * NPH],
                            start=False,
                            stop=True,
                        )
                        nc.any.tensor_copy(t_v[:, nh * NPH : (nh + 1) * NPH], ps[:])
                    vaug_sb.append(t_v)

                # ---- attention per head ----
                at_sb = [
                    wp.tile([128, N], F32R, name=f"at{b}_{j}", tag="at", bufs=12)
                    for j in range(CCH)
                ]
                for h in range(H):
                    qt = qk_sb[h // 2]
                    kt = qk_sb[6 + h // 2]
                    off = (h % 2) * 64
                    # scores transposed: ST[k, q] per k-chunk
                    pt_sb = []
                    for ki, (k0, kl) in enumerate(TCH):
                        q0 = 0 if ki == 0 else 64
                        ps = pp.tile(
                            [kl, N - q0], F32, name=f"pst{b}_{h}_{ki}", tag="pst", bufs=4
                        )
                        nc.tensor.matmul(
                            ps[:],
                            kt[off : off + 64, k0 : k0 + kl],
                            qt[off : off + 64, q0:N],
                            start=True,
                            stop=True,
                        )
                        t_pt = wp.tile(
                            [kl, N - q0], F32R, name=f"pt{b}_{h}_{ki}", tag="pt", bufs=6
                        )
                        nc.scalar.activation(t_pt[:], ps[:], EXP, scale=0.125)
                        pt_sb.append(t_pt)
                    # PV: out rows 0:64 = x_sT unnormalized, row 64 = colsums
                    po = pp.tile([128, N], F32, name=f"po{b}_{h}", tag="po", bufs=2)
                    hs = slice(h * 65, (h + 1) * 65)
                    nc.tensor.matmul(
                        po[0:65, 0:64],
                        vaug_sb[0][0:64, hs],
                        pt_sb[0][0:64, 0:64],
                        start=True,
                        stop=False,
                    )
                    nc.tensor.matmul(
                        po[0:65, 64:N],
                        vaug_sb[0][:, hs],
                        pt_sb[0][:, 64:N],
                        start=False,
                        stop=False,
                    )
                    nc.tensor.matmul(
                        po[0:65, 64:N],
                        vaug_sb[1][:, hs],
                        pt_sb[1][:],
                        start=False,
                        stop=False,
                    )
                    nc.tensor.matmul(
                        po[0:65, 64:N],
                        vaug_sb[2][:, hs],
                        pt_sb[2][:],
                        start=False,
                        stop=True,
                    )
                    # normalize: recip of colsums, broadcast along partitions via PE
                    rcp = wp.tile([1, N], F32R, name=f"rcp{b}_{h}", tag="rcp", bufs=4)
                    with nc.allow_low_precision(reason="fp32r reciprocal"):
                        nc.vector.reciprocal(rcp[:], po[64:65, :])
                    pbc = pp.tile([64, N], F32, name=f"pbc{b}_{h}", tag="pst", bufs=4)
                    nc.tensor.matmul(pbc[:], ones_sb[:, 0:64], rcp[:], start=True, stop=True)
                    xsu = wp.tile([64, N], F32, name=f"xsu{b}_{h}", tag="xsu", bufs=4)
                    nc.any.tensor_copy(xsu[:], po[0:64, :])
                    nc.vector.tensor_mul(
                        at_sb[h // 2][off : off + 64, :], xsu[:], pbc[:]
                    )

                # ---- output projection ----
                for ti, (t0, tl) in enumerate(TCH):
                    t_o = wp.tile([tl, C], F32, name=f"outp{b}_{ti}", tag="outp", bufs=6)
                    for nh in range(2):
                        ps = pp.tile([tl, PH], F32, name=f"psp{b}_{ti}_{nh}", tag="pmm", bufs=2)
                        for c in range(CCH):
                            nc.tensor.matmul(
                                ps[:],
                                at_sb[c][:, t0 : t0 + tl],
                                wp_sb[c][:, nh * PH : (nh + 1) * PH],
                                start=(c == 0),
                                stop=False,
                            )
                        nc.tensor.matmul(
                            ps[:],
                            ones_sb[:, 0:tl],
                            bp_sb[:, nh * PH : (nh + 1) * PH],
                            start=False,
                            stop=True,
                        )
                        nc.any.tensor_copy(t_o[:, nh * PH : (nh + 1) * PH], ps[:])
                    nc.sync.dma_start(d_out[b, t0 : t0 + tl, :], t_o[:])

    nc.compile()
    return nc


def _get_nc():
    if "nc" not in _CACHE:
        _CACHE["nc"] = _build()
    return _CACHE["nc"]


def _host_prep(x, w_qkv, b_qkv, w_proj, b_proj):
    x = np.asarray(x, dtype=np.float32)
    w_qkv = np.asarray(w_qkv, dtype=np.float32)
    b_qkv = np.asarray(b_qkv, dtype=np.float32)
    w_proj = np.asarray(w_proj, dtype=np.float32)
    b_proj = np.asarray(b_proj, dtype=np.float32)

    xt = np.ascontiguousarray(x.transpose(0, 2, 1))  # [B, C, N]
    wqk = np.ascontiguousarray(w_qkv[: 2 * C].T)  # [C, 2C]
    wv_nat = w_qkv[2 * C :]  # [C(hd), C(c)]
    wv = np.zeros((C, VW), dtype=np.float32)
    bv = np.zeros((1, VW), dtype=np.float32)
    for h in range(H):
        wv[:, h * 65 : h * 65 + 64] = wv_nat[h * 64 : (h + 1) * 64].T
        bv[0, h * 65 : h * 65 + 64] = b_qkv[2 * C + h * 64 : 2 * C + (h + 1) * 64]
        bv[0, h * 65 + 64] = 1.0
    bqk = np.ascontiguousarray(b_qkv[: 2 * C].reshape(QK_TILES, 128).T)  # [128, 12]
    wpr = np.ascontiguousarray(w_proj.T)  # [C, C]
    bpr = np.ascontiguousarray(b_proj.reshape(1, C))
    ones = np.ones((1, 128), dtype=np.float32)
    return xt, wqk, wv, wpr, bqk, bv, bpr, ones


def _run(x, w_qkv, b_qkv, w_proj, b_proj, trace=False, trace_cores=None):
    from concourse.bass_utils import run_bass_kernel_spmd

    xt, wqk, wv, wpr, bqk, bv, bpr, ones = _host_prep(x, w_qkv, b_qkv, w_proj, b_proj)
    nc = _get_nc()
    in_maps = []
    for i in range(NCORES):
        in_maps.append(
            {
                "xt": xt[i * BC : (i + 1) * BC],
                "wqk": wqk,
                "wv": wv,
                "wp": wpr,
                "bqk": bqk,
                "bv": bv,
                "bp": bpr,
                "ones": ones,
            }
        )
    kwargs = {}
    if trace:
        kwargs = {"trace": True, "trace_cores": trace_cores or [0]}
    res = run_bass_kernel_spmd(nc, in_maps, core_ids=list(range(NCORES)), **kwargs)
    out = np.concatenate([res.results[i]["out"] for i in range(NCORES)], axis=0)
    return out.astype(np.float32), res


def kernel(x, w_qkv, b_qkv, w_proj, b_proj, num_t, num_s):
    assert int(num_t) == NT and int(num_s) == NS
    out, _ = _run(x, w_qkv, b_qkv, w_proj, b_proj)
    return out


# revision 11
# speedup vs baseline: 1.3079x; 1.3079x over previous
"""Sparse attention (template/search) Trainium2 Bass kernel.

Reference computation (B=64, N=320, C=768, H=12, D=64, num_t=64, num_s=256):
    qkv = x @ w_qkv.T + b_qkv           -> split to q, k, v per head
    template tokens 0:64   attend to tokens 0:64
    search   tokens 64:320 attend to all 320 tokens
    out = attn_out @ w_proj.T + b_proj

Strategy: data-parallel over batch across 8 NeuronCores (8 batches each).
All layout transposes happen on the host (numpy):
  - x is fed transposed (xT [768, 320] per batch) so the contraction dim is on
    SBUF partitions for the qkv projection.
  - w_qkv/w_proj are fed transposed; the v-weights are interleaved per head
    with an extra "ones" column (stride 65) so the PV matmul produces the
    softmax denominators in the same PSUM tile as the attention output.
On-device dataflow per (batch, head):
  STk   = kT[d, kchunk].T @ qT[d, :]        (scores transposed, k on partitions)
  PT    = exp(STk * 0.125)                  (ScalarE, PSUM -> SBUF, fp32r)
  PV    = vaug[k, 65].T @ PT[k, q]          -> [65, 320]: rows 0:64 = attn outT,
                                               row 64 = colsums (ones column)
Normalization is deferred per batch: the 12 heads' colsums are gathered into
one [12, 320] tile, one batched reciprocal, then 12 consecutive PE rank-1
broadcasts (shared stationary operand) + in-place multiplies.
Projection: out[t, co] = aT[c, t].T @ w_projT[c, co] (+bias via rank-1 matmul).
All matmuls use float32r (full PE rate at N>=256, ~1e-4 rel err vs fp32).
Batches are processed in pairs so projection weights stay stationary across
two consecutive matmuls (halves the LDWEIGHTS traffic of the qkv stage).
"""

import sys

sys.path.insert(0, "/opt/trn_rl_repo")

import numpy as np

B, N, C = 64, 320, 768
H, D = 12, 64
NT, NS = 64, 256
NCORES = 8
BC = B // NCORES  # batches per core
CCH = C // 128  # 6 contraction chunks
QK_TILES = (2 * C) // 128  # 12 co-tiles covering q and k sections
TCH = [(0, 128), (128, 128), (256, 64)]  # token chunks (t or k)
VW = H * 65  # 780: v width incl. ones columns
NPH = VW // 2  # 390: vnat free-dim half
PH = C // 2  # 384: proj free-dim half

_CACHE = {}


def _patch_walrus_flags():
    """Enable walrus' redundant-LDWEIGHTS elimination (off by default in this
    toolchain). Verified correct on this kernel by the reference check."""
    import concourse.bass_utils as bu

    if getattr(bu, "_ldw_opt_patched", False):
        return
    orig = bu.run_command

    def patched(argv, **kw):
        argv = [
            a.replace("--enable-ldw-opt=false", "--enable-ldw-opt=true")
            if isinstance(a, str)
            else a
            for a in argv
        ]
        return orig(argv, **kw)

    bu.run_command = patched
    bu._ldw_opt_patched = True


def _build():
    import concourse.bacc as bacc
    import concourse.mybir as mybir
    import concourse.tile as tile

    F32 = mybir.dt.float32
    F32R = mybir.dt.float32r
    EXP = mybir.ActivationFunctionType.Exp

    nc = bacc.Bacc("TRN2")

    d_xt = nc.dram_tensor("xt", [BC, C, N], F32R, kind="ExternalInput")
    d_wqk = nc.dram_tensor("wqk", [C, 2 * C], F32R, kind="ExternalInput")
    d_wv = nc.dram_tensor("wv", [C, VW], F32R, kind="ExternalInput")
    d_wp = nc.dram_tensor("wp", [C, C], F32R, kind="ExternalInput")
    d_bqk = nc.dram_tensor("bqk", [128, QK_TILES], F32, kind="ExternalInput")
    d_bv = nc.dram_tensor("bv", [1, VW], F32R, kind="ExternalInput")
    d_bp = nc.dram_tensor("bp", [1, C], F32R, kind="ExternalInput")
    d_ones = nc.dram_tensor("ones", [1, 128], F32R, kind="ExternalInput")
    d_out = nc.dram_tensor("out", [BC, N, C], F32, kind="ExternalOutput")

    with tile.TileContext(nc) as tc:
        with (
            tc.tile_pool(name="const", bufs=1) as cp,
            tc.tile_pool(name="work", bufs=2) as wp,
            tc.tile_pool(name="psum", bufs=2, space="PSUM") as pp,
        ):
            # ---- resident weights ----
            wqk_sb = []
            wv_sb = []
            wp_sb = []
            for c in range(CCH):
                t_wqk = cp.tile([128, 2 * C], F32R, name=f"wqk{c}", tag=f"wqk{c}")
                nc.sync.dma_start(t_wqk[:], d_wqk[c * 128 : (c + 1) * 128, :])
                wqk_sb.append(t_wqk)
                t_wv = cp.tile([128, VW], F32R, name=f"wv{c}", tag=f"wv{c}")
                nc.sync.dma_start(t_wv[:], d_wv[c * 128 : (c + 1) * 128, :])
                wv_sb.append(t_wv)
                t_wp = cp.tile([128, C], F32R, name=f"wp{c}", tag=f"wp{c}")
                nc.sync.dma_start(t_wp[:], d_wp[c * 128 : (c + 1) * 128, :])
                wp_sb.append(t_wp)
            bqk_sb = cp.tile([128, QK_TILES], F32, name="bqk", tag="bqk")
            nc.sync.dma_start(bqk_sb[:], d_bqk[:])
            bv_sb = cp.tile([1, VW], F32R, name="bv", tag="bv")
            nc.sync.dma_start(bv_sb[:], d_bv[:])
            bp_sb = cp.tile([1, C], F32R, name="bp", tag="bp")
            nc.sync.dma_start(bp_sb[:], d_bp[:])
            ones_sb = cp.tile([1, 128], F32R, name="ones", tag="ones")
            nc.sync.dma_start(ones_sb[:], d_ones[:])

            def attn_batch(b, qk_sb, vaug_sb):
                """Attention for one batch; returns at_sb (normalized attnT)."""
                at_sb = [
                    wp.tile([128, N], F32R, name=f"at{b}_{j}", tag="at", bufs=12)
                    for j in range(CCH)
                ]
                sumsf = wp.tile([1, H * N], F32, name=f"sumsf{b}", tag="sumsf", bufs=1)
                po_list = []
                for h in range(H):
                    qt = qk_sb[h // 2]
                    kt = qk_sb[6 + h // 2]
                    off = (h % 2) * 64
                    # scores transposed: ST[k, q] per k-chunk, exp -> PT
                    pt_sb = []
                    for ki, (k0, kl) in enumerate(TCH):
                        q0 = 0 if ki == 0 else 64
                        ps = pp.tile(
                            [kl, N - q0], F32, name=f"pst{b}_{h}_{ki}", tag="pst", bufs=3
                        )
                        nc.tensor.matmul(
                            ps[:],
                            kt[off : off + 64, k0 : k0 + kl],
                            qt[off : off + 64, q0:N],
                            start=True,
                            stop=True,
                        )
                        t_pt = wp.tile(
                            [kl, N - q0], F32R, name=f"pt{b}_{h}_{ki}", tag="pt", bufs=6
                        )
                        nc.scalar.activation(t_pt[:], ps[:], EXP, scale=0.125)
                        pt_sb.append(t_pt)
                    # PV: rows 0:64 = attn outT (unnormalized), row 64 = colsums
                    po = pp.tile([65, N], F32, name=f"po{b}_{h}", tag="po", bufs=2)
                    hs = slice(h * 65, (h + 1) * 65)
                    nc.tensor.matmul(
                        po[:, 0:64],
                        vaug_sb[0][0:64, hs],
                        pt_sb[0][0:64, 0:64],
                        start=True,
                        stop=False,
                    )
                    nc.tensor.matmul(
                        po[:, 64:N],
                        vaug_sb[0][:, hs],
                        pt_sb[0][:, 64:N],
                        start=False,
                        stop=False,
                    )
                    nc.tensor.matmul(
                        po[:, 64:N],
                        vaug_sb[1][:, hs],
                        pt_sb[1][:],
                        start=False,
                        stop=False,
                    )
                    nc.tensor.matmul(
                        po[:, 64:N],
                        vaug_sb[2][:, hs],
                        pt_sb[2][:],
                        start=False,
                        stop=True,
                    )
                    # evacuate unnormalized rows + colsum row; frees the bank
                    nc.any.tensor_copy(at_sb[h // 2][off : off + 64, :], po[0:64, :])
                    nc.any.tensor_copy(sumsf[0:1, h * N : (h + 1) * N], po[64:65, :])
                    po_list.append(po)
                # batched softmax denominators for all 12 heads
                # scatter the 12 per-head sum rows across partitions (DMA is the
                # only engine free of partition-alignment limits), batch the
                # reciprocal, then flatten back so the broadcast matmuls can
                # slice it at base partition 0
                sums12 = wp.tile([H, N], F32, name=f"sums12{b}", tag="sums12", bufs=1)
                nc.sync.dma_start(sums12[:, :], sumsf[0:1, :])
                rcp = wp.tile([H, N], F32R, name=f"rcp{b}", tag="rcp", bufs=1)
                with nc.allow_low_precision(reason="fp32r reciprocal"):
                    nc.vector.reciprocal(rcp[:], sums12[:])
                rcpf = wp.tile([1, H * N], F32R, name=f"rcpf{b}", tag="rcpf", bufs=1)
                nc.sync.dma_start(rcpf[0:1, :], rcp[:, :])
                # 12 consecutive rank-1 broadcasts share the ones stationary
                for h in range(H):
                    off = (h % 2) * 64
                    pbc = pp.tile([64, N], F32, name=f"pbc{b}_{h}", tag="pst", bufs=3)
                    nc.tensor.matmul(
                        pbc[:],
                        ones_sb[:, 0:64],
                        rcpf[0:1, h * N : (h + 1) * N],
                        start=True,
                        stop=True,
                    )
                    nc.vector.tensor_mul(
                        at_sb[h // 2][off : off + 64, :],
                        at_sb[h // 2][off : off + 64, :],
                        pbc[:],
                    )
                return at_sb

            def proj_batch(b, at_sb):
                for ti, (t0, tl) in enumerate(TCH):
                    t_o = wp.tile([tl, C], F32, name=f"outp{b}_{ti}", tag="outp", bufs=3)
                    ps_h = [
                        pp.tile([tl, PH], F32, name=f"psp{b}_{ti}_{nh}", tag="pmm", bufs=3)
                        for nh in range(2)
                    ]
                    for c in range(CCH):
                        for nh in range(2):
                            nc.tensor.matmul(
                                ps_h[nh][:],
                                at_sb[c][:, t0 : t0 + tl],
                                wp_sb[c][:, nh * PH : (nh + 1) * PH],
                                start=(c == 0),
                                stop=False,
                            )
                    for nh in range(2):
                        nc.tensor.matmul(
                            ps_h[nh][:],
                            ones_sb[:, 0:tl],
                            bp_sb[:, nh * PH : (nh + 1) * PH],
                            start=False,
                            stop=True,
                        )
                    for nh in range(2):
                        nc.any.tensor_copy(t_o[:, nh * PH : (nh + 1) * PH], ps_h[nh][:])
                    nc.sync.dma_start(d_out[b, t0 : t0 + tl, :], t_o[:])

            for bp_i in range(BC // 2):
                bpair = [2 * bp_i, 2 * bp_i + 1]
                xt_sb = {}
                for b in bpair:
                    for c in range(CCH):
                        t_xt = wp.tile(
                            [128, N], F32R, name=f"xt{b}_{c}", tag="xt", bufs=12
                        )
                        nc.sync.dma_start(t_xt[:], d_xt[b, c * 128 : (c + 1) * 128, :])
                        xt_sb[(b, c)] = t_xt

                # ---- qkT projection, batch-paired so each weight tile is
                # stationary across two matmuls ----
                qk_sb = {b: [] for b in bpair}
                for j in range(QK_TILES):
                    ps_b = {
                        b: pp.tile([128, N], F32, name=f"psqk{b}_{j}", tag="pmm", bufs=3)
                        for b in bpair
                    }
                    for c in range(CCH):
                        for b in bpair:
                            nc.tensor.matmul(
                                ps_b[b][:],
                                wqk_sb[c][:, j * 128 : (j + 1) * 128],
                                xt_sb[(b, c)][:],
                                start=(c == 0),
                                stop=(c == CCH - 1),
                            )
                    for b in bpair:
                        t_qk = wp.tile(
                            [128, N], F32R, name=f"qk{b}_{j}", tag="qkt", bufs=24
                        )
                        nc.any.tensor_scalar_add(t_qk[:], ps_b[b][:], bqk_sb[:, j : j + 1])
                        qk_sb[b].append(t_qk)

                # ---- v natural (+ones cols): xT chunk stationary across the
                # two free-dim halves ----
                vaug_sb = {}
                for b in bpair:
                    vlist = []
                    for ti, (t0, tl) in enumerate(TCH):
                        t_v = wp.tile(
                            [tl, VW], F32R, name=f"vaug{b}_{ti}", tag="vaug", bufs=6
                        )
                        ps_h = [
                            pp.tile(
                                [tl, NPH], F32, name=f"psv{b}_{ti}_{nh}", tag="pmm", bufs=3
                            )
                            for nh in range(2)
                        ]
                        for c in range(CCH):
                            for nh in range(2):
                                nc.tensor.matmul(
                                    ps_h[nh][:],
                                    xt_sb[(b, c)][:, t0 : t0 + tl],
                                    wv_sb[c][:, nh * NPH : (nh + 1) * NPH],
                                    start=(c == 0),
                                    stop=False,
                                )
                        for nh in range(2):
                            nc.tensor.matmul(
                                ps_h[nh][:],
                                ones_sb[:, 0:tl],
                                bv_sb[:, nh * NPH : (nh + 1) * NPH],
                                start=False,
                                stop=True,
                            )
                        for nh in range(2):
                            nc.any.tensor_copy(
                                t_v[:, nh * NPH : (nh + 1) * NPH], ps_h[nh][:]
                            )
                        vlist.append(t_v)
                    vaug_sb[b] = vlist

                for b in bpair:
                    at_sb = attn_batch(b, qk_sb[b], vaug_sb[b])
                    proj_batch(b, at_sb)

    nc.compile()
    return nc


def _get_nc():
    if "nc" not in _CACHE:
        _patch_walrus_flags()
        _CACHE["nc"] = _build()
    return _CACHE["nc"]


def _host_prep(x, w_qkv, b_qkv, w_proj, b_proj):
    x = np.asarray(x, dtype=np.float32)
    w_qkv = np.asarray(w_qkv, dtype=np.float32)
    b_qkv = np.asarray(b_qkv, dtype=np.float32)
    w_proj = np.asarray(w_proj, dtype=np.float32)
    b_proj = np.asarray(b_proj, dtype=np.float32)

    xt = np.ascontiguousarray(x.transpose(0, 2, 1))  # [B, C, N]
    wqk = np.ascontiguousarray(w_qkv[: 2 * C].T)  # [C, 2C]
    wv_nat = w_qkv[2 * C :]  # [C(hd), C(c)]
    wv = np.zeros((C, VW), dtype=np.float32)
    bv = np.zeros((1, VW), dtype=np.float32)
    for h in range(H):
        wv[:, h * 65 : h * 65 + 64] = wv_nat[h * 64 : (h + 1) * 64].T
        bv[0, h * 65 : h * 65 + 64] = b_qkv[2 * C + h * 64 : 2 * C + (h + 1) * 64]
        bv[0, h * 65 + 64] = 1.0
    bqk = np.ascontiguousarray(b_qkv[: 2 * C].reshape(QK_TILES, 128).T)  # [128, 12]
    wpr = np.ascontiguousarray(w_proj.T)  # [C, C]
    bpr = np.ascontiguousarray(b_proj.reshape(1, C))
    ones = np.ones((1, 128), dtype=np.float32)
    return xt, wqk, wv, wpr, bqk, bv, bpr, ones


def _run(x, w_qkv, b_qkv, w_proj, b_proj, trace=False, trace_cores=None):
    from concourse.bass_utils import run_bass_kernel_spmd

    xt, wqk, wv, wpr, bqk, bv, bpr, ones = _host_prep(x, w_qkv, b_qkv, w_proj, b_proj)
    nc = _get_nc()
    in_maps = []
    for i in range(NCORES):
        in_maps.append(
            {
                "xt": xt[i * BC : (i + 1) * BC],
                "wqk": wqk,
                "wv": wv,
                "wp": wpr,
                "bqk": bqk,
                "bv": bv,
                "bp": bpr,
                "ones": ones,
            }
        )
    kwargs = {}
    if trace:
        kwargs = {"trace": True, "trace_cores": trace_cores or [0]}
    res = run_bass_kernel_spmd(nc, in_maps, core_ids=list(range(NCORES)), **kwargs)
    out = np.concatenate([res.results[i]["out"] for i in range(NCORES)], axis=0)
    return out.astype(np.float32), res


def kernel(x, w_qkv, b_qkv, w_proj, b_proj, num_t, num_s):
    assert int(num_t) == NT and int(num_s) == NS
    out, _ = _run(x, w_qkv, b_qkv, w_proj, b_proj)
    return out


# revision 14
# speedup vs baseline: 1.3807x; 1.0556x over previous
"""Sparse attention (template/search) Trainium2 Bass kernel.

Reference computation (B=64, N=320, C=768, H=12, D=64, num_t=64, num_s=256):
    qkv = x @ w_qkv.T + b_qkv           -> split to q, k, v per head
    template tokens 0:64   attend to tokens 0:64
    search   tokens 64:320 attend to all 320 tokens
    out = attn_out @ w_proj.T + b_proj

Strategy: data-parallel over batch across 8 NeuronCores (8 batches each).
All layout transposes happen on the host (numpy):
  - x is fed transposed (xT [768, 320] per batch) so the contraction dim is on
    SBUF partitions for the qkv projection.
  - w_qkv/w_proj are fed transposed; the v-weights are interleaved per head
    with an extra "ones" column (stride 65) so the PV matmul produces the
    softmax denominators in the same PSUM tile as the attention output.
On-device dataflow per (batch, head):
  STk   = kT[d, kchunk].T @ qT[d, :]        (scores transposed, k on partitions)
  PT    = exp(STk * 0.125)                  (ScalarE, PSUM -> SBUF, fp32r)
  PV    = vaug[k, 65].T @ PT[k, q]          -> [65, 320]: rows 0:64 = attn outT,
                                               row 64 = colsums (ones column)
Normalization is deferred per batch: the 12 heads' colsums are gathered into
one [12, 320] tile, one batched reciprocal, then 12 consecutive PE rank-1
broadcasts (shared stationary operand) + in-place multiplies.
Projection: out[t, co] = aT[c, t].T @ w_projT[c, co] (+bias via rank-1 matmul).
All matmuls use float32r (full PE rate at N>=256, ~1e-4 rel err vs fp32).
Batches are processed in pairs so projection weights stay stationary across
two consecutive matmuls (halves the LDWEIGHTS traffic of the qkv stage).
"""

import sys

sys.path.insert(0, "/opt/trn_rl_repo")

import numpy as np

B, N, C = 64, 320, 768
H, D = 12, 64
NT, NS = 64, 256
NCORES = 8
BC = B // NCORES  # batches per core
CCH = C // 128  # 6 contraction chunks
QK_TILES = (2 * C) // 128  # 12 co-tiles covering q and k sections
TCH = [(0, 128), (128, 128), (256, 64)]  # token chunks (t or k)
VW = H * 65  # 780: v width incl. ones columns
NPH = VW // 2  # 390: vnat free-dim half
PH = C // 2  # 384: proj free-dim half

_CACHE = {}


def _patch_walrus_flags():
    """Enable walrus' redundant-LDWEIGHTS elimination (off by default in this
    toolchain). Verified correct on this kernel by the reference check."""
    import concourse.bass_utils as bu

    if getattr(bu, "_ldw_opt_patched", False):
        return
    orig = bu.run_command

    def patched(argv, **kw):
        argv = [
            a.replace("--enable-ldw-opt=false", "--enable-ldw-opt=true")
            if isinstance(a, str)
            else a
            for a in argv
        ]
        return orig(argv, **kw)

    bu.run_command = patched
    bu._ldw_opt_patched = True


def _build():
    import concourse.bacc as bacc
    import concourse.mybir as mybir
    import concourse.tile as tile

    F32 = mybir.dt.float32
    F32R = mybir.dt.float32r
    EXP = mybir.ActivationFunctionType.Exp

    nc = bacc.Bacc("TRN2")

    d_xt = nc.dram_tensor("xt", [BC, C, N], F32R, kind="ExternalInput")
    d_wqk = nc.dram_tensor("wqk", [C, 2 * C], F32R, kind="ExternalInput")
    d_wv = nc.dram_tensor("wv", [C, VW], F32R, kind="ExternalInput")
    d_wp = nc.dram_tensor("wp", [C, C], F32R, kind="ExternalInput")
    d_bqk = nc.dram_tensor("bqk", [128, QK_TILES], F32, kind="ExternalInput")
    d_bv = nc.dram_tensor("bv", [1, VW], F32R, kind="ExternalInput")
    d_bp = nc.dram_tensor("bp", [1, C], F32R, kind="ExternalInput")
    d_ones = nc.dram_tensor("ones", [1, 128], F32R, kind="ExternalInput")
    d_out = nc.dram_tensor("out", [BC, N, C], F32, kind="ExternalOutput")

    with tile.TileContext(nc) as tc:
        with (
            tc.tile_pool(name="const", bufs=1) as cp,
            tc.tile_pool(name="work", bufs=2) as wp,
            tc.tile_pool(name="psum", bufs=2, space="PSUM") as pp,
        ):
            # ---- resident weights ----
            wqk_sb = []
            wv_sb = []
            wp_sb = []
            for c in range(CCH):
                t_wqk = cp.tile([128, 2 * C], F32R, name=f"wqk{c}", tag=f"wqk{c}")
                nc.sync.dma_start(t_wqk[:], d_wqk[c * 128 : (c + 1) * 128, :])
                wqk_sb.append(t_wqk)
                t_wv = cp.tile([128, VW], F32R, name=f"wv{c}", tag=f"wv{c}")
                nc.sync.dma_start(t_wv[:], d_wv[c * 128 : (c + 1) * 128, :])
                wv_sb.append(t_wv)
                t_wp = cp.tile([128, C], F32R, name=f"wp{c}", tag=f"wp{c}")
                nc.sync.dma_start(t_wp[:], d_wp[c * 128 : (c + 1) * 128, :])
                wp_sb.append(t_wp)
            bqk_sb = cp.tile([128, QK_TILES], F32, name="bqk", tag="bqk")
            nc.sync.dma_start(bqk_sb[:], d_bqk[:])
            bv_sb = cp.tile([1, VW], F32R, name="bv", tag="bv")
            nc.sync.dma_start(bv_sb[:], d_bv[:])
            bp_sb = cp.tile([1, C], F32R, name="bp", tag="bp")
            nc.sync.dma_start(bp_sb[:], d_bp[:])
            ones_sb = cp.tile([1, 128], F32R, name="ones", tag="ones")
            nc.sync.dma_start(ones_sb[:], d_ones[:])

            def attn_batch(b, qk_sb, vaug_sb):
                """Attention for one batch; returns at_sb (normalized attnT)."""
                at_sb = [
                    wp.tile([128, N], F32R, name=f"at{b}_{j}", tag="at", bufs=12)
                    for j in range(CCH)
                ]
                sumsf = wp.tile([1, H * N], F32, name=f"sumsf{b}", tag="sumsf", bufs=1)
                for hp in range(H // 2):
                    # head pair (2hp, 2hp+1): even head at partitions 0:64, odd
                    # at 64:128 of the same qk tiles. Interleave their score
                    # matmuls so consecutive PE ops hit different row groups.
                    qt = qk_sb[hp]
                    kt = qk_sb[6 + hp]
                    pt_sb = {0: [], 1: []}
                    for ki, (k0, kl) in enumerate(TCH):
                        q0 = 0 if ki == 0 else 64
                        ps_pair = []
                        for par in range(2):
                            off = par * 64
                            ps = pp.tile(
                                [kl, N - q0],
                                F32,
                                name=f"pst{b}_{hp}_{par}_{ki}",
                                tag="pst",
                                bufs=4,
                            )
                            nc.tensor.matmul(
                                ps[:],
                                kt[off : off + 64, k0 : k0 + kl],
                                qt[off : off + 64, q0:N],
                                start=True,
                                stop=True,
                            )
                            ps_pair.append(ps)
                        for par in range(2):
                            t_pt = wp.tile(
                                [kl, N - q0],
                                F32R,
                                name=f"pt{b}_{hp}_{par}_{ki}",
                                tag="pt",
                                bufs=8,
                            )
                            nc.scalar.activation(t_pt[:], ps_pair[par][:], EXP, scale=0.125)
                            pt_sb[par].append(t_pt)
                    for par in range(2):
                        h = 2 * hp + par
                        off = par * 64
                        pts = pt_sb[par]
                        # PV: rows 0:64 = attn outT (unnormalized), row 64 = colsums
                        po = pp.tile([65, N], F32, name=f"po{b}_{h}", tag="po", bufs=2)
                        hs = slice(h * 65, (h + 1) * 65)
                        nc.tensor.matmul(
                            po[:, 0:64],
                            vaug_sb[0][0:64, hs],
                            pts[0][0:64, 0:64],
                            start=True,
                            stop=False,
                        )
                        nc.tensor.matmul(
                            po[:, 64:N],
                            vaug_sb[0][:, hs],
                            pts[0][:, 64:N],
                            start=False,
                            stop=False,
                        )
                        nc.tensor.matmul(
                            po[:, 64:N], vaug_sb[1][:, hs], pts[1][:], start=False, stop=False
                        )
                        nc.tensor.matmul(
                            po[:, 64:N], vaug_sb[2][:, hs], pts[2][:], start=False, stop=True
                        )
                        # evacuate unnormalized rows + colsum row; frees the bank
                        nc.vector.tensor_copy(at_sb[h // 2][off : off + 64, :], po[0:64, :])
                        nc.vector.tensor_copy(sumsf[0:1, h * N : (h + 1) * N], po[64:65, :])
                # batched softmax denominators for all 12 heads
                # scatter the 12 per-head sum rows across partitions (DMA is the
                # only engine free of partition-alignment limits), batch the
                # reciprocal, then flatten back so the broadcast matmuls can
                # slice it at base partition 0
                sums12 = wp.tile([H, N], F32, name=f"sums12{b}", tag="sums12", bufs=1)
                nc.sync.dma_start(sums12[:, :], sumsf[0:1, :])
                rcp = wp.tile([H, N], F32R, name=f"rcp{b}", tag="rcp", bufs=1)
                with nc.allow_low_precision(reason="fp32r reciprocal"):
                    nc.vector.reciprocal(rcp[:], sums12[:])
                rcpf = wp.tile([1, H * N], F32R, name=f"rcpf{b}", tag="rcpf", bufs=1)
                nc.sync.dma_start(rcpf[0:1, :], rcp[:, :])
                # 12 consecutive rank-1 broadcasts share the ones stationary
                for h in range(H):
                    off = (h % 2) * 64
                    pbc = pp.tile([64, N], F32, name=f"pbc{b}_{h}", tag="pst", bufs=4)
                    nc.tensor.matmul(
                        pbc[:],
                        ones_sb[:, 0:64],
                        rcpf[0:1, h * N : (h + 1) * N],
                        start=True,
                        stop=True,
                    )
                    nc.vector.tensor_mul(
                        at_sb[h // 2][off : off + 64, :],
                        at_sb[h // 2][off : off + 64, :],
                        pbc[:],
                    )
                return at_sb

            def proj_batch(b, at_sb):
                for ti, (t0, tl) in enumerate(TCH):
                    t_o = wp.tile([tl, C], F32, name=f"outp{b}_{ti}", tag="outp", bufs=2)
                    ps_h = [
                        pp.tile([tl, PH], F32, name=f"psp{b}_{ti}_{nh}", tag="pmm", bufs=2)
                        for nh in range(2)
                    ]
                    for c in range(CCH):
                        for nh in range(2):
                            nc.tensor.matmul(
                                ps_h[nh][:],
                                at_sb[c][:, t0 : t0 + tl],
                                wp_sb[c][:, nh * PH : (nh + 1) * PH],
                                start=(c == 0),
                                stop=False,
                            )
                    for nh in range(2):
                        nc.tensor.matmul(
                            ps_h[nh][:],
                            ones_sb[:, 0:tl],
                            bp_sb[:, nh * PH : (nh + 1) * PH],
                            start=False,
                            stop=True,
                        )
                    for nh in range(2):
                        nc.any.tensor_copy(t_o[:, nh * PH : (nh + 1) * PH], ps_h[nh][:])
                    nc.sync.dma_start(d_out[b, t0 : t0 + tl, :], t_o[:])

            for bp_i in range(BC // 2):
                bpair = [2 * bp_i, 2 * bp_i + 1]
                xt_sb = {}
                for b in bpair:
                    for c in range(CCH):
                        t_xt = wp.tile(
                            [128, N], F32R, name=f"xt{b}_{c}", tag="xt", bufs=12
                        )
                        nc.sync.dma_start(t_xt[:], d_xt[b, c * 128 : (c + 1) * 128, :])
                        xt_sb[(b, c)] = t_xt

                # ---- qkT projection, batch-paired so each weight tile is
                # stationary across two matmuls ----
                qk_sb = {b: [] for b in bpair}
                for j in range(QK_TILES):
                    ps_b = {
                        b: pp.tile([128, N], F32, name=f"psqk{b}_{j}", tag="pmm", bufs=2)
                        for b in bpair
                    }
                    for c in range(CCH):
                        for b in bpair:
                            nc.tensor.matmul(
                                ps_b[b][:],
                                wqk_sb[c][:, j * 128 : (j + 1) * 128],
                                xt_sb[(b, c)][:],
                                start=(c == 0),
                                stop=(c == CCH - 1),
                            )
                    for b in bpair:
                        t_qk = wp.tile(
                            [128, N], F32R, name=f"qk{b}_{j}", tag="qkt", bufs=24
                        )
                        nc.any.tensor_scalar_add(t_qk[:], ps_b[b][:], bqk_sb[:, j : j + 1])
                        qk_sb[b].append(t_qk)

                # ---- v natural (+ones cols): xT chunk stationary across the
                # two free-dim halves ----
                vaug_sb = {}
                for b in bpair:
                    vlist = []
                    for ti, (t0, tl) in enumerate(TCH):
                        t_v = wp.tile(
                            [tl, VW], F32R, name=f"vaug{b}_{ti}", tag="vaug", bufs=6
                        )
                        ps_h = [
                            pp.tile(
                                [tl, NPH], F32, name=f"psv{b}_{ti}_{nh}", tag="pmm", bufs=2
                            )
                            for nh in range(2)
                        ]
                        for c in range(CCH):
                            for nh in range(2):
                                nc.tensor.matmul(
                                    ps_h[nh][:],
                                    xt_sb[(b, c)][:, t0 : t0 + tl],
                                    wv_sb[c][:, nh * NPH : (nh + 1) * NPH],
                                    start=(c == 0),
                                    stop=False,
                                )
                        for nh in range(2):
                            nc.tensor.matmul(
                                ps_h[nh][:],
                                ones_sb[:, 0:tl],
                                bv_sb[:, nh * NPH : (nh + 1) * NPH],
                                start=False,
                                stop=True,
                            )
                        for nh in range(2):
                            nc.any.tensor_copy(
                                t_v[:, nh * NPH : (nh + 1) * NPH], ps_h[nh][:]
                            )
                        vlist.append(t_v)
                    vaug_sb[b] = vlist

                for b in bpair:
                    at_sb = attn_batch(b, qk_sb[b], vaug_sb[b])
                    proj_batch(b, at_sb)

    nc.compile()
    return nc


def _get_nc():
    if "nc" not in _CACHE:
        _patch_walrus_flags()
        _CACHE["nc"] = _build()
    return _CACHE["nc"]


def _host_prep(x, w_qkv, b_qkv, w_proj, b_proj):
    x = np.asarray(x, dtype=np.float32)
    w_qkv = np.asarray(w_qkv, dtype=np.float32)
    b_qkv = np.asarray(b_qkv, dtype=np.float32)
    w_proj = np.asarray(w_proj, dtype=np.float32)
    b_proj = np.asarray(b_proj, dtype=np.float32)

    xt = np.ascontiguousarray(x.transpose(0, 2, 1))  # [B, C, N]
    wqk = np.ascontiguousarray(w_qkv[: 2 * C].T)  # [C, 2C]
    wv_nat = w_qkv[2 * C :]  # [C(hd), C(c)]
    wv = np.zeros((C, VW), dtype=np.float32)
    bv = np.zeros((1, VW), dtype=np.float32)
    for h in range(H):
        wv[:, h * 65 : h * 65 + 64] = wv_nat[h * 64 : (h + 1) * 64].T
        bv[0, h * 65 : h * 65 + 64] = b_qkv[2 * C + h * 64 : 2 * C + (h + 1) * 64]
        bv[0, h * 65 + 64] = 1.0
    bqk = np.ascontiguousarray(b_qkv[: 2 * C].reshape(QK_TILES, 128).T)  # [128, 12]
    wpr = np.ascontiguousarray(w_proj.T)  # [C, C]
    bpr = np.ascontiguousarray(b_proj.reshape(1, C))
    ones = np.ones((1, 128), dtype=np.float32)
    return xt, wqk, wv, wpr, bqk, bv, bpr, ones


def _run(x, w_qkv, b_qkv, w_proj, b_proj, trace=False, trace_cores=None):
    from concourse.bass_utils import run_bass_kernel_spmd

    xt, wqk, wv, wpr, bqk, bv, bpr, ones = _host_prep(x, w_qkv, b_qkv, w_proj, b_proj)
    nc = _get_nc()
    in_maps = []
    for i in range(NCORES):
        in_maps.append(
            {
                "xt": xt[i * BC : (i + 1) * BC],
                "wqk": wqk,
                "wv": wv,
                "wp": wpr,
                "bqk": bqk,
                "bv": bv,
                "bp": bpr,
                "ones": ones,
            }
        )
    kwargs = {}
    if trace:
        kwargs = {"trace": True, "trace_cores": trace_cores or [0]}
    res = run_bass_kernel_spmd(nc, in_maps, core_ids=list(range(NCORES)), **kwargs)
    out = np.concatenate([res.results[i]["out"] for i in range(NCORES)], axis=0)
    return out.astype(np.float32), res


def kernel(x, w_qkv, b_qkv, w_proj, b_proj, num_t, num_s):
    assert int(num_t) == NT and int(num_s) == NS
    out, _ = _run(x, w_qkv, b_qkv, w_proj, b_proj)
    return out


# revision 17
# speedup vs baseline: 1.4797x; 1.0717x over previous
"""Sparse attention (template/search) Trainium2 Bass kernel.

Reference computation (B=64, N=320, C=768, H=12, D=64, num_t=64, num_s=256):
    qkv = x @ w_qkv.T + b_qkv           -> split to q, k, v per head
    template tokens 0:64   attend to tokens 0:64
    search   tokens 64:320 attend to all 320 tokens
    out = attn_out @ w_proj.T + b_proj

Strategy: data-parallel over batch across 8 NeuronCores (8 batches each).
All layout transposes happen on the host (numpy):
  - x is fed transposed (xT [768, 320] per batch) so the contraction dim is on
    SBUF partitions for the qkv projection.
  - w_qkv/w_proj are fed transposed; the v-weights are interleaved per head
    with an extra "ones" column (stride 65) so the PV matmul produces the
    softmax denominators in the same PSUM tile as the attention output.
On-device dataflow per (batch, head):
  STk   = kT[d, kchunk].T @ qT[d, :]        (scores transposed, k on partitions)
  PT    = exp(STk * 0.125)                  (ScalarE, PSUM -> SBUF, fp32r)
  PV    = vaug[k, 65].T @ PT[k, q]          -> [65, 320]: rows 0:64 = attn outT,
                                               row 64 = colsums (ones column)
Normalization is deferred per batch: the 12 heads' colsums are gathered into
one [12, 320] tile, one batched reciprocal, then 12 consecutive PE rank-1
broadcasts (shared stationary operand) + in-place multiplies.
Projection: out[t, co] = aT[c, t].T @ w_projT[c, co] (+bias via rank-1 matmul).
All matmuls use float32r (full PE rate at N>=256, ~1e-4 rel err vs fp32).
Batches are processed in pairs so projection weights stay stationary across
two consecutive matmuls (halves the LDWEIGHTS traffic of the qkv stage).
"""

import sys

sys.path.insert(0, "/opt/trn_rl_repo")

import numpy as np

B, N, C = 64, 320, 768
H, D = 12, 64
NT, NS = 64, 256
NCORES = 8
BC = B // NCORES  # batches per core
CCH = C // 128  # 6 contraction chunks
QK_TILES = (2 * C) // 128  # 12 co-tiles covering q and k sections
TCH = [(0, 128), (128, 128), (256, 64)]  # token chunks (t or k)
VW = H * 65  # 780: v width incl. ones columns
NPH = VW // 2  # 390: vnat free-dim half
PH = C // 2  # 384: proj free-dim half

_CACHE = {}


def _patch_walrus_flags():
    """Enable walrus' redundant-LDWEIGHTS elimination (off by default in this
    toolchain). Verified correct on this kernel by the reference check."""
    import concourse.bass_utils as bu

    if getattr(bu, "_ldw_opt_patched", False):
        return
    orig = bu.run_command

    def patched(argv, **kw):
        argv = [
            a.replace("--enable-ldw-opt=false", "--enable-ldw-opt=true")
            if isinstance(a, str)
            else a
            for a in argv
        ]
        return orig(argv, **kw)

    bu.run_command = patched
    bu._ldw_opt_patched = True


def _build():
    import concourse.bacc as bacc
    import concourse.mybir as mybir
    import concourse.tile as tile

    F32 = mybir.dt.float32
    F32R = mybir.dt.float32r
    EXP = mybir.ActivationFunctionType.Exp

    nc = bacc.Bacc("TRN2")

    d_xt = nc.dram_tensor("xt", [BC, C, N], F32R, kind="ExternalInput")
    d_wqk = nc.dram_tensor("wqk", [C, 2 * C], F32R, kind="ExternalInput")
    d_wv = nc.dram_tensor("wv", [C, VW], F32R, kind="ExternalInput")
    d_wp = nc.dram_tensor("wp", [C, C], F32R, kind="ExternalInput")
    d_bqk = nc.dram_tensor("bqk", [128, QK_TILES], F32, kind="ExternalInput")
    d_bv = nc.dram_tensor("bv", [1, VW], F32R, kind="ExternalInput")
    d_bp = nc.dram_tensor("bp", [1, C], F32R, kind="ExternalInput")
    d_ones = nc.dram_tensor("ones", [1, 128], F32R, kind="ExternalInput")
    d_out = nc.dram_tensor("out", [BC, N, C], F32, kind="ExternalOutput")

    with tile.TileContext(nc) as tc:
        with (
            tc.tile_pool(name="const", bufs=1) as cp,
            tc.tile_pool(name="work", bufs=2) as wp,
            tc.tile_pool(name="psum", bufs=2, space="PSUM") as pp,
        ):
            # ---- resident weights ----
            wqk_sb = []
            wv_sb = []
            wp_sb = []
            for c in range(CCH):
                t_wqk = cp.tile([128, 2 * C], F32R, name=f"wqk{c}", tag=f"wqk{c}")
                nc.sync.dma_start(t_wqk[:], d_wqk[c * 128 : (c + 1) * 128, :])
                wqk_sb.append(t_wqk)
                t_wv = cp.tile([128, VW], F32R, name=f"wv{c}", tag=f"wv{c}")
                nc.sync.dma_start(t_wv[:], d_wv[c * 128 : (c + 1) * 128, :])
                wv_sb.append(t_wv)
                t_wp = cp.tile([128, C], F32R, name=f"wp{c}", tag=f"wp{c}")
                nc.sync.dma_start(t_wp[:], d_wp[c * 128 : (c + 1) * 128, :])
                wp_sb.append(t_wp)
            bqk_sb = cp.tile([128, QK_TILES], F32, name="bqk", tag="bqk")
            nc.sync.dma_start(bqk_sb[:], d_bqk[:])
            bv_sb = cp.tile([1, VW], F32R, name="bv", tag="bv")
            nc.sync.dma_start(bv_sb[:], d_bv[:])
            bp_sb = cp.tile([1, C], F32R, name="bp", tag="bp")
            nc.sync.dma_start(bp_sb[:], d_bp[:])
            ones_sb = cp.tile([1, 128], F32R, name="ones", tag="ones")
            nc.sync.dma_start(ones_sb[:], d_ones[:])

            def attn_batch(b, qk_sb, vaug_sb):
                """Attention for one batch; returns at_sb (normalized attnT)."""
                at_sb = [
                    wp.tile([128, N], F32R, name=f"at{b}_{j}", tag="at", bufs=12)
                    for j in range(CCH)
                ]
                sumsf = wp.tile([1, H * N], F32, name=f"sumsf{b}", tag="sumsf", bufs=1)
                for hp in range(H // 2):
                    # head pair (2hp, 2hp+1): even head at partitions 0:64, odd
                    # at 64:128 of the same qk tiles. Interleave their score
                    # matmuls so consecutive PE ops hit different row groups.
                    qt = qk_sb[hp]
                    kt = qk_sb[6 + hp]
                    pt_sb = {0: [], 1: []}
                    for ki, (k0, kl) in enumerate(TCH):
                        q0 = 0 if ki == 0 else 64
                        ps_pair = []
                        for par in range(2):
                            off = par * 64
                            ps = pp.tile(
                                [kl, N - q0],
                                F32,
                                name=f"pst{b}_{hp}_{par}_{ki}",
                                tag="pst",
                                bufs=4,
                            )
                            nc.tensor.matmul(
                                ps[:],
                                kt[off : off + 64, k0 : k0 + kl],
                                qt[off : off + 64, q0:N],
                                start=True,
                                stop=True,
                            )
                            ps_pair.append(ps)
                        for par in range(2):
                            t_pt = wp.tile(
                                [kl, N - q0],
                                F32R,
                                name=f"pt{b}_{hp}_{par}_{ki}",
                                tag="pt",
                                bufs=7,
                            )
                            nc.scalar.activation(t_pt[:], ps_pair[par][:], EXP, scale=0.125)
                            pt_sb[par].append(t_pt)
                    for par in range(2):
                        h = 2 * hp + par
                        off = par * 64
                        pts = pt_sb[par]
                        # PV: rows 0:64 = attn outT (unnormalized), row 64 = colsums
                        po = pp.tile([65, N], F32, name=f"po{b}_{h}", tag="po", bufs=2)
                        hs = slice(h * 65, (h + 1) * 65)
                        nc.tensor.matmul(
                            po[:, 0:64],
                            vaug_sb[0][0:64, hs],
                            pts[0][0:64, 0:64],
                            start=True,
                            stop=False,
                        )
                        nc.tensor.matmul(
                            po[:, 64:N],
                            vaug_sb[0][:, hs],
                            pts[0][:, 64:N],
                            start=False,
                            stop=False,
                        )
                        nc.tensor.matmul(
                            po[:, 64:N], vaug_sb[1][:, hs], pts[1][:], start=False, stop=False
                        )
                        nc.tensor.matmul(
                            po[:, 64:N], vaug_sb[2][:, hs], pts[2][:], start=False, stop=True
                        )
                        # evacuate unnormalized rows + colsum row; frees the bank
                        nc.vector.tensor_copy(at_sb[h // 2][off : off + 64, :], po[0:64, :])
                        nc.vector.tensor_copy(sumsf[0:1, h * N : (h + 1) * N], po[64:65, :])
                # batched softmax denominators for all 12 heads:
                # scatter the 12 per-head sum rows across partitions (DMA is the
                # only engine free of partition-alignment limits), batch the
                # reciprocal, then flatten back so the broadcast matmuls can
                # slice it at base partition 0
                sums12 = wp.tile([H, N], F32, name=f"sums12{b}", tag="sums12", bufs=2)
                nc.sync.dma_start(sums12[:, :], sumsf[0:1, :])
                rcp = wp.tile([H, N], F32R, name=f"rcp{b}", tag="rcp", bufs=2)
                with nc.allow_low_precision(reason="fp32r reciprocal"):
                    nc.vector.reciprocal(rcp[:], sums12[:])
                rcpf = wp.tile([1, H * N], F32R, name=f"rcpf{b}", tag="rcpf", bufs=1)
                nc.sync.dma_start(rcpf[0:1, :], rcp[:, :])
                return at_sb, rcpf

            def norm_batch(b, at_sb, rcpf):
                # 12 consecutive rank-1 broadcasts share the ones stationary
                for h in range(H):
                    off = (h % 2) * 64
                    pbc = pp.tile([64, N], F32, name=f"pbc{b}_{h}", tag="pst", bufs=4)
                    nc.tensor.matmul(
                        pbc[:],
                        ones_sb[:, 0:64],
                        rcpf[0:1, h * N : (h + 1) * N],
                        start=True,
                        stop=True,
                    )
                    nc.vector.tensor_mul(
                        at_sb[h // 2][off : off + 64, :],
                        at_sb[h // 2][off : off + 64, :],
                        pbc[:],
                    )

            def proj_batch(b, at_sb):
                for ti, (t0, tl) in enumerate(TCH):
                    t_o = wp.tile([tl, C], F32, name=f"outp{b}_{ti}", tag="outp", bufs=2)
                    ps_h = [
                        pp.tile([tl, PH], F32, name=f"psp{b}_{ti}_{nh}", tag="pmm", bufs=2)
                        for nh in range(2)
                    ]
                    for c in range(CCH):
                        for nh in range(2):
                            nc.tensor.matmul(
                                ps_h[nh][:],
                                at_sb[c][:, t0 : t0 + tl],
                                wp_sb[c][:, nh * PH : (nh + 1) * PH],
                                start=(c == 0),
                                stop=False,
                            )
                    for nh in range(2):
                        nc.tensor.matmul(
                            ps_h[nh][:],
                            ones_sb[:, 0:tl],
                            bp_sb[:, nh * PH : (nh + 1) * PH],
                            start=False,
                            stop=True,
                        )
                    for nh in range(2):
                        nc.any.tensor_copy(t_o[:, nh * PH : (nh + 1) * PH], ps_h[nh][:])
                    nc.sync.dma_start(d_out[b, t0 : t0 + tl, :], t_o[:])

            for bp_i in range(BC // 2):
                bpair = [2 * bp_i, 2 * bp_i + 1]
                xt_sb = {}
                for b in bpair:
                    for c in range(CCH):
                        t_xt = wp.tile(
                            [128, N], F32R, name=f"xt{b}_{c}", tag="xt", bufs=12
                        )
                        nc.sync.dma_start(t_xt[:], d_xt[b, c * 128 : (c + 1) * 128, :])
                        xt_sb[(b, c)] = t_xt

                # ---- qkT projection, batch-paired so each weight tile is
                # stationary across two matmuls ----
                qk_sb = {b: [] for b in bpair}
                for j in range(QK_TILES):
                    ps_b = {
                        b: pp.tile([128, N], F32, name=f"psqk{b}_{j}", tag="pmm", bufs=2)
                        for b in bpair
                    }
                    for c in range(CCH):
                        for b in bpair:
                            nc.tensor.matmul(
                                ps_b[b][:],
                                wqk_sb[c][:, j * 128 : (j + 1) * 128],
                                xt_sb[(b, c)][:],
                                start=(c == 0),
                                stop=(c == CCH - 1),
                            )
                    for b in bpair:
                        t_qk = wp.tile(
                            [128, N], F32R, name=f"qk{b}_{j}", tag="qkt", bufs=24
                        )
                        nc.any.tensor_scalar_add(t_qk[:], ps_b[b][:], bqk_sb[:, j : j + 1])
                        qk_sb[b].append(t_qk)

                # ---- v natural (+ones cols): xT chunk stationary across the
                # two free-dim halves ----
                vaug_sb = {}
                for b in bpair:
                    vlist = []
                    for ti, (t0, tl) in enumerate(TCH):
                        t_v = wp.tile(
                            [tl, VW], F32R, name=f"vaug{b}_{ti}", tag="vaug", bufs=6
                        )
                        ps_h = [
                            pp.tile(
                                [tl, NPH], F32, name=f"psv{b}_{ti}_{nh}", tag="pmm", bufs=2
                            )
                            for nh in range(2)
                        ]
                        for c in range(CCH):
                            for nh in range(2):
                                nc.tensor.matmul(
                                    ps_h[nh][:],
                                    xt_sb[(b, c)][:, t0 : t0 + tl],
                                    wv_sb[c][:, nh * NPH : (nh + 1) * NPH],
                                    start=(c == 0),
                                    stop=False,
                                )
                        for nh in range(2):
                            nc.tensor.matmul(
                                ps_h[nh][:],
                                ones_sb[:, 0:tl],
                                bv_sb[:, nh * NPH : (nh + 1) * NPH],
                                start=False,
                                stop=True,
                            )
                        for nh in range(2):
                            nc.any.tensor_copy(
                                t_v[:, nh * NPH : (nh + 1) * NPH], ps_h[nh][:]
                            )
                        vlist.append(t_v)
                    vaug_sb[b] = vlist

                # both batches' attention first, then normalize+project: the
                # ~9us sums->reciprocal->broadcast latency chain of one batch
                # hides under the other batch's score/PV matmuls
                res = {}
                for b in bpair:
                    res[b] = attn_batch(b, qk_sb[b], vaug_sb[b])
                for b in bpair:
                    norm_batch(b, *res[b])
                    proj_batch(b, res[b][0])

    nc.compile()
    return nc


def _get_nc():
    if "nc" not in _CACHE:
        _patch_walrus_flags()
        _CACHE["nc"] = _build()
    return _CACHE["nc"]


def _host_prep(x, w_qkv, b_qkv, w_proj, b_proj):
    x = np.asarray(x, dtype=np.float32)
    w_qkv = np.asarray(w_qkv, dtype=np.float32)
    b_qkv = np.asarray(b_qkv, dtype=np.float32)
    w_proj = np.asarray(w_proj, dtype=np.float32)
    b_proj = np.asarray(b_proj, dtype=np.float32)

    xt = np.ascontiguousarray(x.transpose(0, 2, 1))  # [B, C, N]
    wqk = np.ascontiguousarray(w_qkv[: 2 * C].T)  # [C, 2C]
    wv_nat = w_qkv[2 * C :]  # [C(hd), C(c)]
    wv = np.zeros((C, VW), dtype=np.float32)
    bv = np.zeros((1, VW), dtype=np.float32)
    for h in range(H):
        wv[:, h * 65 : h * 65 + 64] = wv_nat[h * 64 : (h + 1) * 64].T
        bv[0, h * 65 : h * 65 + 64] = b_qkv[2 * C + h * 64 : 2 * C + (h + 1) * 64]
        bv[0, h * 65 + 64] = 1.0
    bqk = np.ascontiguousarray(b_qkv[: 2 * C].reshape(QK_TILES, 128).T)  # [128, 12]
    wpr = np.ascontiguousarray(w_proj.T)  # [C, C]
    bpr = np.ascontiguousarray(b_proj.reshape(1, C))
    ones = np.ones((1, 128), dtype=np.float32)
    return xt, wqk, wv, wpr, bqk, bv, bpr, ones


def _run(x, w_qkv, b_qkv, w_proj, b_proj, trace=False, trace_cores=None):
    from concourse.bass_utils import run_bass_kernel_spmd

    xt, wqk, wv, wpr, bqk, bv, bpr, ones = _host_prep(x, w_qkv, b_qkv, w_proj, b_proj)
    nc = _get_nc()
    in_maps = []
    for i in range(NCORES):
        in_maps.append(
            {
                "xt": xt[i * BC : (i + 1) * BC],
                "wqk": wqk,
                "wv": wv,
                "wp": wpr,
                "bqk": bqk,
                "bv": bv,
                "bp": bpr,
                "ones": ones,
            }
        )
    kwargs = {}
    if trace:
        kwargs = {"trace": True, "trace_cores": trace_cores or [0]}
    res = run_bass_kernel_spmd(nc, in_maps, core_ids=list(range(NCORES)), **kwargs)
    out = np.concatenate([res.results[i]["out"] for i in range(NCORES)], axis=0)
    return out.astype(np.float32), res


def kernel(x, w_qkv, b_qkv, w_proj, b_proj, num_t, num_s):
    assert int(num_t) == NT and int(num_s) == NS
    out, _ = _run(x, w_qkv, b_qkv, w_proj, b_proj)
    return out
